# revision 33
# baseline (speedup 1.0000x reference)
"""Trainium2 Bass kernel for causal multi-head attention (B=2, T=2048, C=1024, H=16, D=64).

Sharding (8 NeuronCores): data-parallel over the 2 batches x tensor-parallel over
4 head-groups (4 heads each). Core c handles batch c//4, heads 4*(c%4)..4*(c%4)+3.
Each core computes its 4 heads' QKV projections, causal attention, and a partial
output projection against its slice of Wo's columns; the host sums the 4 partials
per batch (the row-parallel unshard).

v4 design notes. The PE matmul stream is ~102us at 2.4GHz and is the floor for
this decomposition (projections 55us + S 14.5us + AV 29us + l-broadcast 3.4us);
ACT is second at ~79us of softmax EXPs (measured: 0.837ns/col + 259ns/instr).
v2 (161us) lost ~44us to PE issue stalls plus ~20us of head/tail; this version
measures ~153us at full clock. NOTE: the device package-power/DVFS state is
shared — a noisy neighbor can depress the whole-chip clock ~1.2x uniformly
(check exp-duration-vs-size against the 0.837ns/col model before trusting any
measurement). Scheduling:
  - Per-unit step loop emits S(p) then AV(p-2). The 2-deep psum_s ring already
    forces S(p+2) to wait for exp(p), so by the time AV(p) is issued its exp is
    provably complete: AV never stalls on ACT.
  - All deferrable projection work (qk rounds for t>=1024, V tiles 8-15, Y
    ranges 0-2) is queued as ~0.4-1.7us chunks and drip-fed one per step inside
    the later units, where ACT (exp) outpaces the unit's own S+AV work. Each
    unit's chunk list is sized to its ACT deficit and respects readiness:
    qk(1,0) drains inside (1,1) [needed by (2,0)], qk(1,1) inside (2,0),
    v12-15 inside (2,1), Y range r needs units (r,*) normalized.
  - Diagonal tiles run 3rd..6th in each unit so their exp->gpsimd-mask->AV
    chain (~1.7us of engine-hop latency) hides behind 4+ steps of other work.
  - Warmup matmuls (DVFS ramp: cold PE is ~4x slower, ramp takes 3us) stream a
    memset scrap with no readers (WAW slot rotation is PE-internal), so they
    start as soon as the first gpsimd memset lands instead of after the full
    constants chain; 22 of them bridge the DMA wait. Dependency-free scrap
    matmuls are also padded into the first DMA-chased projection groups
    (skip_group_check interleave) so the 3us continuous-busy ramp is never
    reset by an x-chunk gap — full 2.4GHz arrives at ~18us instead of ~23us.
  - Tail: the last Y range runs on a 4-deep psum ring (psum_p plus the by-then
    idle psum_s banks) with evictions alternating DVE/ACT, so the final 8
    projection groups pipeline instead of serializing on a 2-ring. The first 4
    groups' kc=0 matmuls (contracting only the hk=0 oT block, normalized one
    unit earlier) are prefetched under the final normalize chain.
  - All oT evictions stay on the DVE: the late phase is exp-bound on ACT, so
    ACT-side copies there cost wall time (ACT copies only in the exp-free
    tail).
  - The softmax normalize chain is DMA-free and gpsimd-free: l rows are PE-
    broadcast via K=1 selector matmuls into PSUM, 1/l comes from the DVE
    (reciprocal_approx_fast, ~18 bits), and one full-height multiply
    normalizes a unit. Normalize multiplies are deferred one unit (except the
    last) so the DVE stream never head-of-line blocks on the recip chain.
  - Y is written bf16 (host sums partials in fp32; ~0.1% extra error).
  - PSUM budget is exactly 8 banks: scores 2x[128,2,512] + AV pair 2x[128,512]
    + projections 2x[128,512].

Device algorithm (per core), all matmuls bf16 with fp32 PSUM accumulation:
  QT = WqS @ x^T            [256, T]   (d-major; head pair per 128-row block)
  KT = WkS @ x^T            [256, T]
  V  = x @ WvS^T            [T, 4, 66] (t-major, head-strided, ones column at 64)
  per q-tile i (512 wide), head-pair hk, key-tile j (128 wide, j <= 4i+3):
    S^T both heads -> one [128, 2, 512] PSUM tile via two row-group-packed
      matmuls (K=64 each, tile_position (0,0) and (64,0))
    P^T = exp(S^T / 8)      (one ACT op per pair tile; unsafe softmax)
    P^T *= tri-mask         (diagonal blocks; gpsimd)
    O^T_h[:, i] += V_h[j]^T @ P^T_h   (M=65: ones column accumulates l)
  per (i, hk): l rows PE-broadcast via K=1 selector matmuls (psum_o ring), rec
    = 1/l on the DVE, one in-place scale of O^T;
    then Y[i-range] = O^T-as-lhsT @ WoS^T -> DRAM (bf16)
"""

import numpy as np

B, T, C = 2, 2048, 1024
H, D = 16, 64
HPC = 4  # heads per core
N_CORES = 8
DH = HPC * D  # 256: per-core projection width

_compiled = None


def _patch_act_tables():
    """Make Exp and Ln resolve to the one table set that holds both, so the
    softmax exps never thrash ACT_TABLE_LOADs."""
    import functools

    import concourse.hw_specs as hw_specs
    import concourse.mybir as mybir
    from concourse import bacc

    if getattr(bacc, "_act_tables_patched", False):
        return
    orig = hw_specs.get_activation_tables

    @functools.cache
    def patched(arch):
        tabs = {k: set(v) for k, v in orig(arch).items()}
        E = mybir.ActivationFunctionType.Exp
        L = mybir.ActivationFunctionType.Ln
        keep = "natural_log_exp_and_others"
        if keep in tabs and E in tabs[keep] and L in tabs[keep]:
            for name, fns in tabs.items():
                if name != keep:
                    fns.discard(E)
                    fns.discard(L)
        return tabs

    bacc.get_activation_tables = patched
    bacc._act_tables_patched = True


def _build():
    import concourse.bass as bass
    import concourse.mybir as mybir
    from concourse import bacc
    from concourse.tile import TileContext

    _patch_act_tables()

    dt = mybir.dt
    BF = dt.bfloat16
    F32 = dt.float32
    ts = bass.ts
    Act = mybir.ActivationFunctionType

    P = 128
    NQ = T // 512   # 4 q-tiles of 512
    NK = T // 128   # 16 key-tiles of 128
    KC = C // 128   # 8 contraction subtiles for the projections

    nc = bacc.Bacc("TRN2", target_bir_lowering=False, debug=False)

    xT_d = nc.dram_tensor("xT", [C, T], BF, kind="ExternalInput")
    wq_d = nc.dram_tensor("wqT", [C, DH], BF, kind="ExternalInput")
    wk_d = nc.dram_tensor("wkT", [C, DH], BF, kind="ExternalInput")
    wv_d = nc.dram_tensor("wvT", [C, DH], BF, kind="ExternalInput")
    wo_d = nc.dram_tensor("woT", [DH, C], BF, kind="ExternalInput")
    y_d = nc.dram_tensor("y", [T, C], BF, kind="ExternalOutput")

    xT_r = xT_d[:].rearrange("(ko p) (nq t) -> p ko nq t", p=P, nq=NQ)
    wq_r = wq_d[:].rearrange("(ko p) m -> p ko m", p=P)
    wk_r = wk_d[:].rearrange("(ko p) m -> p ko m", p=P)
    wv_r = wv_d[:].rearrange("(ko p) m -> p ko m", p=P)
    wo_r = wo_d[:].rearrange("(ko p) n -> p ko n", p=P)

    with TileContext(nc) as tc:
        with (
            tc.tile_pool(name="persist", bufs=1) as persist,
            tc.tile_pool(name="ptiles", bufs=28) as ptiles,
            tc.tile_pool(name="ytiles", bufs=8) as ytiles,
            tc.tile_pool(name="ltmp", bufs=4) as ltmp,
            tc.tile_pool(name="rbpool", bufs=4) as rbpool,
            tc.tile_pool(name="psum_s", bufs=2, space="PSUM") as psum_s,
            tc.tile_pool(name="psum_o", bufs=2, space="PSUM") as psum_o,
            tc.tile_pool(name="psum_p", bufs=2, space="PSUM") as psum_p,
        ):
            # ---- persistent SBUF tensors -------------------------------
            x_sb = persist.tile([P, KC, T], BF, tag="x")          # x^T
            wq_sb = persist.tile([P, KC, DH], BF, tag="wq")
            wk_sb = persist.tile([P, KC, DH], BF, tag="wk")
            wv_sb = persist.tile([P, KC, DH], BF, tag="wv")
            wo_sb = persist.tile([P, DH // P, C], BF, tag="wo")
            qT_sb = persist.tile([P, DH // P, T], BF, tag="qT")
            kT_sb = persist.tile([P, DH // P, T], BF, tag="kT")
            v_sb = persist.tile([P, NK, HPC, 66], BF, tag="v")
            oT_sb = persist.tile([P, DH // P, T], BF, tag="oT")
            cmask = persist.tile([P, 2, P], BF, tag="cmask")
            sel_t = persist.tile([P, 2, P], BF, tag="sel_t")
            scrap = persist.tile([P, 256], BF, tag="scrap")

            # ---- DMAs in consumption order -----------------------------
            nc.sync.dma_start(wq_sb[:, :, 0:P], wq_r[:, :, 0:P])
            for ko in range(KC):
                nc.sync.dma_start(x_sb[:, ko, ts(0, 512)], xT_r[:, ko, 0:1, :])
            nc.sync.dma_start(wq_sb[:, :, P:DH], wq_r[:, :, P:DH])
            nc.sync.dma_start(wk_sb[:], wk_r)
            nc.sync.dma_start(wv_sb[:], wv_r)
            for ko in range(KC):
                nc.sync.dma_start(x_sb[:, ko, ts(1, 512)], xT_r[:, ko, 1:2, :])
            for ko in range(KC):
                nc.sync.dma_start(x_sb[:, ko, ts(2, 512)], xT_r[:, ko, 2:3, :])
            for ko in range(KC):
                nc.sync.dma_start(x_sb[:, ko, ts(3, 512)], xT_r[:, ko, 3:4, :])
            nc.sync.dma_start(wo_sb[:], wo_r)

            # ---- DVFS pre-warm ----------------------------------------
            # dummy matmuls on a freshly-memset scrap bridge the window
            # until the first x/wq DMAs land, so the PE's 3us clock ramp
            # completes BEFORE the real projections start. No readers:
            # the psum_s slot WAW rotation is PE-internal ordering.
            nc.gpsimd.memset(scrap[:], 0.0)
            # 22 warmups: ~14 at mid-pstate complete the 3us ramp, the
            # rest keep the PE continuously busy until the first x/wq DMAs
            # land (~11.5us) so the projections start at 2.4GHz instead of
            # re-ramping from 1.2GHz until ~23us
            for w in range(22):
                pw = psum_s.tile([P, 2, 512], F32, tag="s")
                nc.tensor.matmul(
                    pw[:, 0, 0:256],
                    scrap[:, 0:P],
                    scrap[:],
                    start=True,
                    stop=True,
                )

            # ---- constants: causal corner mask + V's ones column -------
            nc.gpsimd.memset(v_sb[:, :, :, 64:66], 1.0)
            # selector rows for the l-broadcast matmuls: the two K=1
            # accumulating matmuls route half h's l to out rows 64h..64h+63
            nc.gpsimd.memset(sel_t[:], 0.0)
            nc.gpsimd.memset(sel_t[64:65, 0, 0:64], 1.0)
            nc.gpsimd.memset(sel_t[64:65, 1, 64:128], 1.0)
            # causal corner mask: keep 1.0 where col >= row, else 0.0
            nc.gpsimd.memset(cmask[:], 1.0)
            for half in range(2):
                nc.gpsimd.affine_select(
                    out=cmask[:, half, :],
                    in_=cmask[:, half, :],
                    compare_op=mybir.AluOpType.is_ge,
                    fill=0.0,
                    base=0,
                    pattern=[[1, P]],
                    channel_multiplier=-1,
                )

            # ---- projection chunk generators ---------------------------
            # qk_group: one [128,512] psum group of the QT/KT projection
            # (w in {q,k}, m-half, 512-wide t-range); v_round: one V t-tile;
            # y_group: one (mt, n) group of the output projection.
            def scrap_mm():
                # DVFS ramp keep-alive: a dependency-free matmul that fills
                # a DMA-chase gap so the 3us continuous-busy ramp to 2.4GHz
                # is not reset (the early phase otherwise sits at 1.2GHz
                # until ~23us)
                pw = psum_s.tile([P, 2, 512], F32, tag="s")
                nc.tensor.matmul(
                    pw[:, 0, 0:256], scrap[:, 0:P], scrap[:],
                    start=True, stop=True, skip_group_check=True,
                )

            def qk_group(w_sb, out_sb, m, r, pad=False):
                ps = psum_p.tile([P, 512], F32, tag="pp")
                for k in range(KC):
                    nc.tensor.matmul(
                        ps[:],
                        w_sb[:, k, ts(m, P)],
                        x_sb[:, k, ts(r, 512)],
                        start=(k == 0),
                        stop=(k == KC - 1),
                        skip_group_check=pad,
                    )
                    if pad and k % 2 == 1 and k < KC - 1:
                        scrap_mm()
                nc.vector.tensor_copy(out_sb[:, m, ts(r, 512)], ps[:])

            def qk_round(r, pad=False):
                for w_sb, out_sb in ((wq_sb, qT_sb), (wk_sb, kT_sb)):
                    for m in range(DH // P):
                        qk_group(w_sb, out_sb, m, r, pad=pad)

            def v_round(mt, pad=False):
                ps_full = psum_p.tile([P, 512], F32, tag="pp")
                ps = ps_full[:, :DH]
                for k in range(KC):
                    nc.tensor.matmul(
                        ps[:],
                        x_sb[:, k, ts(mt, P)],
                        wv_sb[:, k, :],
                        start=(k == 0),
                        stop=(k == KC - 1),
                        skip_group_check=pad,
                    )
                    if pad and k % 2 == 1 and k < KC - 1:
                        scrap_mm()
                nc.vector.tensor_copy(
                    v_sb[:, mt, :, 0:64], ps.rearrange("p (h d) -> p h d", d=64)
                )

            def y_group(mt, n, pool=None, evict="v", dma=None):
                py_full = (pool or psum_p).tile(
                    [P, 2, 512] if pool is psum_s else [P, 512], F32,
                    tag="s" if pool is psum_s else "pp",
                )
                py = py_full[:, 0, :] if pool is psum_s else py_full[:]
                for kc in range(DH // P):
                    nc.tensor.matmul(
                        py[:],
                        oT_sb[:, kc, ts(mt, P)],
                        wo_sb[:, kc, ts(n, 512)],
                        start=(kc == 0),
                        stop=(kc == DH // P - 1),
                    )
                yt = ytiles.tile([P, 512], BF, tag="y")
                if evict == "s":
                    nc.scalar.copy(yt[:], py[:])
                else:
                    nc.vector.tensor_copy(yt[:], py[:])
                (dma or nc.sync).dma_start(y_d[ts(mt, P), ts(n, 512)], yt[:])

            # ---- static filler assignment ------------------------------
            # chunks drip-fed one per step inside each unit (i, hk); sized
            # to the unit's ACT-over-PE deficit, ordered by readiness.
            def mk_qk_chunks(r):
                return [
                    (lambda w=w_sb, o=out_sb, m=m, r=r: qk_group(w, o, m, r))
                    for w_sb, out_sb in ((wq_sb, qT_sb), (wk_sb, kT_sb))
                    for m in range(DH // P)
                ]

            def mk_y_chunks(r, lo, hi):
                return [
                    (lambda mt=mt, n=n: y_group(mt, n))
                    for g in range(lo, hi)
                    for mt, n in [(4 * r + g // 2, g % 2)]
                ]

            qk01 = mk_qk_chunks(1)
            qk10 = mk_qk_chunks(2)
            qk11 = mk_qk_chunks(3)
            fillers = {
                (0, 0): qk01[0:2],
                (0, 1): qk01[2:4],
                (1, 0): [lambda: v_round(8), lambda: v_round(9)],
                (1, 1): qk10,
                (2, 0): qk11,
                (2, 1): [lambda mt=mt: v_round(mt) for mt in range(12, 16)]
                + mk_y_chunks(0, 0, 2),
                (3, 0): mk_y_chunks(0, 2, 8) + mk_y_chunks(1, 0, 4),
                (3, 1): mk_y_chunks(1, 4, 8) + mk_y_chunks(2, 0, 8),
            }

            # ---- attention units --------------------------------------
            pending_mults = []

            def emit_unit(i, hk):
                nonlocal pending_mults
                # flush the previous unit's deferred normalizes first:
                # their rec inputs are surely ready, and placing them
                # ahead of this unit's recip chain in the DVE stream
                # keeps a lagging chain from head-of-line blocking them
                for args in pending_mults:
                    nc.vector.tensor_mul(*args)
                pending_mults = []

                jmax = 4 * i + 3
                nd = list(range(4 * i))
                diag = list(range(4 * i, 4 * i + 4))
                # diag 3rd..6th: their exp->mask->AV chain gets 4+ steps
                jlist = nd[:2] + diag + nd[2:]
                chunks = list(fillers.get((i, hk), []))
                pts = {}
                op0 = psum_o.tile([P, 512], F32, tag="o")
                op1 = psum_o.tile([P, 512], F32, tag="o")
                ops = [op0, op1]

                def emit_s(j):
                    c0 = P * (j - 4 * i) if j >= 4 * i else 0
                    sp = psum_s.tile([P, 2, 512], F32, tag="s")
                    for half in range(2):
                        hp = 64 * half
                        nc.tensor.matmul(
                            sp[:, half, c0:],
                            kT_sb[hp : hp + 64, hk, ts(j, P)],
                            qT_sb[hp : hp + 64, hk, 512 * i + c0 : 512 * (i + 1)],
                            start=True,
                            stop=True,
                            tile_position=(hp, 0),
                        )
                    pt = ptiles.tile([P, 2, 512], BF, tag="p")
                    if j >= 4 * i:
                        t = j - 4 * i
                        nc.scalar.activation(
                            pt[:, :, P * t :], sp[:, :, P * t :],
                            Act.Exp, scale=0.125,
                        )
                        nc.gpsimd.tensor_mul(
                            pt[:, :, P * t : P * (t + 1)],
                            pt[:, :, P * t : P * (t + 1)],
                            cmask[:],
                        )
                    else:
                        nc.scalar.activation(pt[:], sp[:], Act.Exp, scale=0.125)
                    pts[j] = pt

                def emit_av(pos):
                    j = jlist[pos]
                    c0 = P * (j - 4 * i) if j >= 4 * i else 0
                    for half in range(2):
                        h = 2 * hk + half
                        nc.tensor.matmul(
                            ops[half][0:65, c0:],
                            v_sb[:, j, h, 0:65],
                            pts[j][:, half, c0:],
                            start=(pos == 0),
                            stop=(pos == len(jlist) - 1),
                        )

                # step loop: filler, S(p), AV(p-2). The psum_s 2-ring makes
                # S(p) wait on exp(p-2), so AV(p-2)'s input is provably
                # ready when it issues.
                n = len(jlist)
                for p in range(n):
                    if p > 0 and chunks:
                        chunks.pop(0)()
                    emit_s(jlist[p])
                    if p >= 2:
                        emit_av(p - 2)
                for p in (n - 2, n - 1):
                    if chunks:
                        chunks.pop(0)()
                    emit_av(p)
                while chunks:
                    chunks.pop(0)()

                # evict O^T (unnormalized) + l rows, then normalize.
                lt = ltmp.tile([P, 2, 512], BF, tag="lt")
                for half in range(2):
                    nc.vector.tensor_copy(
                        lt[64:65, half, :], ops[half][64:65, :]
                    )
                nc.vector.tensor_copy(
                    oT_sb[0:64, hk, ts(i, 512)], ops[0][0:64, :]
                )
                nc.vector.tensor_copy(
                    oT_sb[64:128, hk, ts(i, 512)], ops[1][0:64, :]
                )
                # NOT the psum_p ring: FIFO slot rotation there would
                # make later projection groups wait on this unit's recip
                # chain, serializing the PE filler behind attention
                lb = psum_o.tile([P, 512], F32, tag="o")
                for half in range(2):
                    nc.tensor.matmul(
                        lb[:],
                        sel_t[64:65, half, :],
                        lt[64:65, half, :],
                        start=(half == 0),
                        stop=(half == 1),
                    )
                rec = rbpool.tile([P, 512], F32, tag="rec")
                nc.vector.reciprocal_approx_fast(rec[:], lb[:])
                mult = (
                    oT_sb[:, hk, ts(i, 512)],
                    oT_sb[:, hk, ts(i, 512)],
                    rec[:],
                )
                if i == NQ - 1 and hk == DH // P - 1:
                    nc.vector.tensor_mul(*mult)
                else:
                    pending_mults.append(mult)

            # ---- emission ---------------------------------------------
            qk_round(0, pad=True)
            for mt in range(4):
                v_round(mt, pad=(mt < 2))
            emit_unit(0, 0)
            emit_unit(0, 1)
            for mt in range(4, 8):
                v_round(mt)
            emit_unit(1, 0)
            emit_unit(1, 1)
            for mt in range(10, 12):
                v_round(mt)
            emit_unit(2, 0)
            emit_unit(2, 1)
            emit_unit(3, 0)
            emit_unit(3, 1)

            # tail: Y range 3 on a 4-deep psum ring (psum_p + the now-idle
            # psum_s banks), evictions alternating DVE/ACT. The first 4
            # groups' kc=0 matmuls read only the hk=0 oT block (normalized
            # at unit (3,1) start), so they run while the final normalize
            # chain (lt->lb->recip->mult) is still landing.
            tail_py = []
            for g in range(4):
                mt, n = 12 + g // 2, g % 2
                if g < 2:
                    pg = psum_p.tile([P, 512], F32, tag="pp")
                    py = pg[:]
                else:
                    pg = psum_s.tile([P, 2, 512], F32, tag="s")
                    py = pg[:, 0, :]
                nc.tensor.matmul(
                    py[:],
                    oT_sb[:, 0, ts(mt, P)],
                    wo_sb[:, 0, ts(n, 512)],
                    start=True,
                    stop=False,
                    skip_group_check=True,
                )
                tail_py.append((py, mt, n))
            for g, (py, mt, n) in enumerate(tail_py):
                nc.tensor.matmul(
                    py[:],
                    oT_sb[:, 1, ts(mt, P)],
                    wo_sb[:, 1, ts(n, 512)],
                    start=False,
                    stop=True,
                    skip_group_check=True,
                )
                yt = ytiles.tile([P, 512], BF, tag="y")
                if g % 2:
                    nc.scalar.copy(yt[:], py[:])
                    nc.gpsimd.dma_start(y_d[ts(mt, P), ts(n, 512)], yt[:])
                else:
                    nc.vector.tensor_copy(yt[:], py[:])
                    nc.scalar.dma_start(y_d[ts(mt, P), ts(n, 512)], yt[:])
            for g in range(4, 8):
                mt, n = 12 + g // 2, g % 2
                pool = psum_p if (g // 2) % 2 == 0 else psum_s
                y_group(
                    mt, n, pool=pool, evict="s" if g % 2 else "v",
                    dma=nc.gpsimd if g % 2 else nc.scalar,
                )

    nc.compile()
    return nc


def _get_compiled():
    global _compiled
    if _compiled is None:
        _compiled = _build()
    return _compiled


def make_inputs(x, Wq, Wk, Wv, Wo):
    """Shard the full inputs into the 8 per-core input maps (host-side prep)."""
    import ml_dtypes

    bf16 = ml_dtypes.bfloat16
    x = np.asarray(x)
    in_maps = []
    for c in range(N_CORES):
        b, g = divmod(c, HPC)
        rows = slice(g * DH, (g + 1) * DH)
        in_maps.append(
            {
                "xT": np.ascontiguousarray(x[b].T).astype(bf16),
                "wqT": np.ascontiguousarray(np.asarray(Wq)[rows, :].T).astype(bf16),
                "wkT": np.ascontiguousarray(np.asarray(Wk)[rows, :].T).astype(bf16),
                "wvT": np.ascontiguousarray(np.asarray(Wv)[rows, :].T).astype(bf16),
                "woT": np.ascontiguousarray(np.asarray(Wo)[:, rows].T).astype(bf16),
            }
        )
    return in_maps


def assemble(results):
    """Sum the 4 tensor-parallel partials per batch into the full output."""
    y = np.zeros((B, T, C), dtype=np.float32)
    for c in range(N_CORES):
        b = c // HPC
        y[b] += np.asarray(results[c]["y"]).astype(np.float32)
    return y


def kernel(x, Wq, Wk, Wv, Wo):
    from concourse.bass_utils import run_bass_kernel_spmd

    nc = _get_compiled()
    in_maps = make_inputs(x, Wq, Wk, Wv, Wo)
    res = run_bass_kernel_spmd(nc, in_maps, list(range(N_CORES)))
    return assemble(res.results)


# revision 34
# speedup vs baseline: 1.0078x; 1.0078x over previous
"""Trainium2 Bass kernel for causal multi-head attention (B=2, T=2048, C=1024, H=16, D=64).

Sharding (8 NeuronCores): data-parallel over the 2 batches x tensor-parallel over
4 head-groups (4 heads each). Core c handles batch c//4, heads 4*(c%4)..4*(c%4)+3.
Each core computes its 4 heads' QKV projections, causal attention, and a partial
output projection against its slice of Wo's columns; the host sums the 4 partials
per batch (the row-parallel unshard).

v4 design notes. The PE matmul stream is ~102us at 2.4GHz and is the floor for
this decomposition (projections 55us + S 14.5us + AV 29us + l-broadcast 3.4us);
ACT is second at ~79us of softmax EXPs (measured: 0.837ns/col + 259ns/instr).
v2 (161us) lost ~44us to PE issue stalls plus ~20us of head/tail; this version
measures ~153us at full clock. NOTE: the device package-power/DVFS state is
shared — a noisy neighbor can depress the whole-chip clock ~1.2x uniformly
(check exp-duration-vs-size against the 0.837ns/col model before trusting any
measurement). Scheduling:
  - Per-unit step loop emits S(p) then AV(p-2). The 2-deep psum_s ring already
    forces S(p+2) to wait for exp(p), so by the time AV(p) is issued its exp is
    provably complete: AV never stalls on ACT.
  - All deferrable projection work (qk rounds for t>=1024, V tiles 8-15, Y
    ranges 0-2) is queued as ~0.4-1.7us chunks and drip-fed one per step inside
    the later units, where ACT (exp) outpaces the unit's own S+AV work. Each
    unit's chunk list is sized to its ACT deficit and respects readiness:
    qk(1,0) drains inside (1,1) [needed by (2,0)], qk(1,1) inside (2,0),
    v12-15 inside (2,1), Y range r needs units (r,*) normalized.
  - Diagonal tiles run 3rd..6th in each unit so their exp->gpsimd-mask->AV
    chain (~1.7us of engine-hop latency) hides behind 4+ steps of other work.
  - Warmup matmuls (DVFS ramp: cold PE is ~4x slower, ramp takes 3us) stream a
    memset scrap with no readers (WAW slot rotation is PE-internal), so they
    start as soon as the first gpsimd memset lands instead of after the full
    constants chain; 22 of them bridge the DMA wait. Dependency-free scrap
    matmuls are also padded into the first DMA-chased projection groups
    (skip_group_check interleave) so the 3us continuous-busy ramp is never
    reset by an x-chunk gap — full 2.4GHz arrives at ~18us instead of ~23us.
  - Tail: the last Y range runs on a 4-deep psum ring (psum_p plus the by-then
    idle psum_s banks) with evictions alternating DVE/ACT, so the final 8
    projection groups pipeline instead of serializing on a 2-ring. The first 4
    groups' kc=0 matmuls (contracting only the hk=0 oT block, normalized one
    unit earlier) are prefetched under the final normalize chain.
  - All oT evictions stay on the DVE: the late phase is exp-bound on ACT, so
    ACT-side copies there cost wall time (ACT copies only in the exp-free
    tail).
  - The softmax normalize chain is DMA-free and gpsimd-free: l rows are PE-
    broadcast via K=1 selector matmuls into PSUM, 1/l comes from the DVE
    (reciprocal_approx_fast, ~18 bits), and one full-height multiply
    normalizes a unit. Normalize multiplies are deferred one unit (except the
    last) so the DVE stream never head-of-line blocks on the recip chain.
  - Y is written bf16 (host sums partials in fp32; ~0.1% extra error).
  - PSUM budget is exactly 8 banks: scores 2x[128,2,512] + AV pair 2x[128,512]
    + projections 2x[128,512].

Device algorithm (per core), all matmuls bf16 with fp32 PSUM accumulation:
  QT = WqS @ x^T            [256, T]   (d-major; head pair per 128-row block)
  KT = WkS @ x^T            [256, T]
  V  = x @ WvS^T            [T, 4, 66] (t-major, head-strided, ones column at 64)
  per q-tile i (512 wide), head-pair hk, key-tile j (128 wide, j <= 4i+3):
    S^T both heads -> one [128, 2, 512] PSUM tile via two row-group-packed
      matmuls (K=64 each, tile_position (0,0) and (64,0))
    P^T = exp(S^T / 8)      (one ACT op per pair tile; unsafe softmax)
    P^T *= tri-mask         (diagonal blocks; gpsimd)
    O^T_h[:, i] += V_h[j]^T @ P^T_h   (M=65: ones column accumulates l)
  per (i, hk): l rows PE-broadcast via K=1 selector matmuls (psum_o ring), rec
    = 1/l on the DVE, one in-place scale of O^T;
    then Y[i-range] = O^T-as-lhsT @ WoS^T -> DRAM (bf16)
"""

import numpy as np

B, T, C = 2, 2048, 1024
H, D = 16, 64
HPC = 4  # heads per core
N_CORES = 8
DH = HPC * D  # 256: per-core projection width

_compiled = None


def _patch_act_tables():
    """Make Exp and Ln resolve to the one table set that holds both, so the
    softmax exps never thrash ACT_TABLE_LOADs."""
    import functools

    import concourse.hw_specs as hw_specs
    import concourse.mybir as mybir
    from concourse import bacc

    if getattr(bacc, "_act_tables_patched", False):
        return
    orig = hw_specs.get_activation_tables

    @functools.cache
    def patched(arch):
        tabs = {k: set(v) for k, v in orig(arch).items()}
        E = mybir.ActivationFunctionType.Exp
        L = mybir.ActivationFunctionType.Ln
        keep = "natural_log_exp_and_others"
        if keep in tabs and E in tabs[keep] and L in tabs[keep]:
            for name, fns in tabs.items():
                if name != keep:
                    fns.discard(E)
                    fns.discard(L)
        return tabs

    bacc.get_activation_tables = patched
    bacc._act_tables_patched = True


def _build():
    import concourse.bass as bass
    import concourse.mybir as mybir
    from concourse import bacc
    from concourse.tile import TileContext

    _patch_act_tables()

    dt = mybir.dt
    BF = dt.bfloat16
    F32 = dt.float32
    ts = bass.ts
    Act = mybir.ActivationFunctionType

    P = 128
    NQ = T // 512   # 4 q-tiles of 512
    NK = T // 128   # 16 key-tiles of 128
    KC = C // 128   # 8 contraction subtiles for the projections

    nc = bacc.Bacc("TRN2", target_bir_lowering=False, debug=False)

    xT_d = nc.dram_tensor("xT", [C, T], BF, kind="ExternalInput")
    wq_d = nc.dram_tensor("wqT", [C, DH], BF, kind="ExternalInput")
    wk_d = nc.dram_tensor("wkT", [C, DH], BF, kind="ExternalInput")
    wv_d = nc.dram_tensor("wvT", [C, DH], BF, kind="ExternalInput")
    wo_d = nc.dram_tensor("woT", [DH, C], BF, kind="ExternalInput")
    y_d = nc.dram_tensor("y", [T, C], BF, kind="ExternalOutput")

    xT_r = xT_d[:].rearrange("(ko p) (nq t) -> p ko nq t", p=P, nq=NQ)
    wq_r = wq_d[:].rearrange("(ko p) m -> p ko m", p=P)
    wk_r = wk_d[:].rearrange("(ko p) m -> p ko m", p=P)
    wv_r = wv_d[:].rearrange("(ko p) m -> p ko m", p=P)
    wo_r = wo_d[:].rearrange("(ko p) n -> p ko n", p=P)

    with TileContext(nc) as tc:
        with (
            tc.tile_pool(name="persist", bufs=1) as persist,
            tc.tile_pool(name="ptiles", bufs=28) as ptiles,
            tc.tile_pool(name="ytiles", bufs=8) as ytiles,
            tc.tile_pool(name="ltmp", bufs=4) as ltmp,
            tc.tile_pool(name="rbpool", bufs=4) as rbpool,
            tc.tile_pool(name="psum_s", bufs=2, space="PSUM") as psum_s,
            tc.tile_pool(name="psum_o", bufs=2, space="PSUM") as psum_o,
            tc.tile_pool(name="psum_p", bufs=2, space="PSUM") as psum_p,
        ):
            # ---- persistent SBUF tensors -------------------------------
            x_sb = persist.tile([P, KC, T], BF, tag="x")          # x^T
            wq_sb = persist.tile([P, KC, DH], BF, tag="wq")
            wk_sb = persist.tile([P, KC, DH], BF, tag="wk")
            wv_sb = persist.tile([P, KC, DH], BF, tag="wv")
            wo_sb = persist.tile([P, DH // P, C], BF, tag="wo")
            qT_sb = persist.tile([P, DH // P, T], BF, tag="qT")
            kT_sb = persist.tile([P, DH // P, T], BF, tag="kT")
            v_sb = persist.tile([P, NK, HPC, 66], BF, tag="v")
            oT_sb = persist.tile([P, DH // P, T], BF, tag="oT")
            cmask = persist.tile([P, 2, P], BF, tag="cmask")
            sel_t = persist.tile([P, 2, P], BF, tag="sel_t")
            scrap = persist.tile([P, 256], BF, tag="scrap")

            # ---- DMAs in consumption order -----------------------------
            nc.sync.dma_start(wq_sb[:, :, 0:P], wq_r[:, :, 0:P])
            for ko in range(KC):
                nc.sync.dma_start(x_sb[:, ko, ts(0, 512)], xT_r[:, ko, 0:1, :])
            nc.sync.dma_start(wq_sb[:, :, P:DH], wq_r[:, :, P:DH])
            nc.sync.dma_start(wk_sb[:], wk_r)
            nc.sync.dma_start(wv_sb[:], wv_r)
            for ko in range(KC):
                nc.sync.dma_start(x_sb[:, ko, ts(1, 512)], xT_r[:, ko, 1:2, :])
            for ko in range(KC):
                nc.sync.dma_start(x_sb[:, ko, ts(2, 512)], xT_r[:, ko, 2:3, :])
            for ko in range(KC):
                nc.sync.dma_start(x_sb[:, ko, ts(3, 512)], xT_r[:, ko, 3:4, :])
            nc.sync.dma_start(wo_sb[:], wo_r)

            # ---- DVFS pre-warm ----------------------------------------
            # dummy matmuls on a freshly-memset scrap bridge the window
            # until the first x/wq DMAs land, so the PE's 3us clock ramp
            # completes BEFORE the real projections start. No readers:
            # the psum_s slot WAW rotation is PE-internal ordering.
            nc.gpsimd.memset(scrap[:], 0.0)
            # 22 warmups: ~14 at mid-pstate complete the 3us ramp, the
            # rest keep the PE continuously busy until the first x/wq DMAs
            # land (~11.5us) so the projections start at 2.4GHz instead of
            # re-ramping from 1.2GHz until ~23us
            for w in range(22):
                pw = psum_s.tile([P, 2, 512], F32, tag="s")
                nc.tensor.matmul(
                    pw[:, 0, 0:256],
                    scrap[:, 0:P],
                    scrap[:],
                    start=True,
                    stop=True,
                )

            # ---- constants: causal corner mask + V's ones column -------
            nc.gpsimd.memset(v_sb[:, :, :, 64:66], 1.0)
            # selector rows for the l-broadcast matmuls: the two K=1
            # accumulating matmuls route half h's l to out rows 64h..64h+63
            nc.gpsimd.memset(sel_t[:], 0.0)
            nc.gpsimd.memset(sel_t[64:65, 0, 0:64], 1.0)
            nc.gpsimd.memset(sel_t[64:65, 1, 64:128], 1.0)
            # causal corner mask: keep 1.0 where col >= row, else 0.0
            nc.gpsimd.memset(cmask[:], 1.0)
            for half in range(2):
                nc.gpsimd.affine_select(
                    out=cmask[:, half, :],
                    in_=cmask[:, half, :],
                    compare_op=mybir.AluOpType.is_ge,
                    fill=0.0,
                    base=0,
                    pattern=[[1, P]],
                    channel_multiplier=-1,
                )

            # ---- projection chunk generators ---------------------------
            # qk_group: one [128,512] psum group of the QT/KT projection
            # (w in {q,k}, m-half, 512-wide t-range); v_round: one V t-tile;
            # y_group: one (mt, n) group of the output projection.
            def scrap_mm():
                # DVFS ramp keep-alive: a dependency-free matmul that fills
                # a DMA-chase gap so the 3us continuous-busy ramp to 2.4GHz
                # is not reset (the early phase otherwise sits at 1.2GHz
                # until ~23us)
                pw = psum_s.tile([P, 2, 512], F32, tag="s")
                nc.tensor.matmul(
                    pw[:, 0, 0:256], scrap[:, 0:P], scrap[:],
                    start=True, stop=True, skip_group_check=True,
                )

            def qk_group(w_sb, out_sb, m, r, pad=False):
                ps = psum_p.tile([P, 512], F32, tag="pp")
                for k in range(KC):
                    nc.tensor.matmul(
                        ps[:],
                        w_sb[:, k, ts(m, P)],
                        x_sb[:, k, ts(r, 512)],
                        start=(k == 0),
                        stop=(k == KC - 1),
                        skip_group_check=pad,
                    )
                    if pad and k % 2 == 1 and k < KC - 1:
                        scrap_mm()
                nc.vector.tensor_copy(out_sb[:, m, ts(r, 512)], ps[:])

            def qk_round(r, pad=False):
                for w_sb, out_sb in ((wq_sb, qT_sb), (wk_sb, kT_sb)):
                    for m in range(DH // P):
                        qk_group(w_sb, out_sb, m, r, pad=pad)

            def v_round(mt, pad=False):
                ps_full = psum_p.tile([P, 512], F32, tag="pp")
                ps = ps_full[:, :DH]
                for k in range(KC):
                    nc.tensor.matmul(
                        ps[:],
                        x_sb[:, k, ts(mt, P)],
                        wv_sb[:, k, :],
                        start=(k == 0),
                        stop=(k == KC - 1),
                        skip_group_check=pad,
                    )
                    if pad and k % 2 == 1 and k < KC - 1:
                        scrap_mm()
                nc.vector.tensor_copy(
                    v_sb[:, mt, :, 0:64], ps.rearrange("p (h d) -> p h d", d=64)
                )

            def y_group(mt, n, pool=None, evict="v"):
                py_full = (pool or psum_p).tile(
                    [P, 2, 512] if pool is psum_s else [P, 512], F32,
                    tag="s" if pool is psum_s else "pp",
                )
                py = py_full[:, 0, :] if pool is psum_s else py_full[:]
                for kc in range(DH // P):
                    nc.tensor.matmul(
                        py[:],
                        oT_sb[:, kc, ts(mt, P)],
                        wo_sb[:, kc, ts(n, 512)],
                        start=(kc == 0),
                        stop=(kc == DH // P - 1),
                    )
                yt = ytiles.tile([P, 512], BF, tag="y")
                if evict == "s":
                    nc.scalar.copy(yt[:], py[:])
                else:
                    nc.vector.tensor_copy(yt[:], py[:])
                nc.sync.dma_start(y_d[ts(mt, P), ts(n, 512)], yt[:])

            # ---- static filler assignment ------------------------------
            # chunks drip-fed one per step inside each unit (i, hk); sized
            # to the unit's ACT-over-PE deficit, ordered by readiness.
            def mk_qk_chunks(r):
                return [
                    (lambda w=w_sb, o=out_sb, m=m, r=r: qk_group(w, o, m, r))
                    for w_sb, out_sb in ((wq_sb, qT_sb), (wk_sb, kT_sb))
                    for m in range(DH // P)
                ]

            def mk_y_chunks(r, lo, hi):
                return [
                    (lambda mt=mt, n=n: y_group(mt, n))
                    for g in range(lo, hi)
                    for mt, n in [(4 * r + g // 2, g % 2)]
                ]

            qk01 = mk_qk_chunks(1)
            qk10 = mk_qk_chunks(2)
            qk11 = mk_qk_chunks(3)
            fillers = {
                (0, 0): qk01[0:2],
                (0, 1): qk01[2:4],
                (1, 0): [lambda: v_round(8), lambda: v_round(9)],
                (1, 1): qk10,
                (2, 0): qk11,
                (2, 1): [lambda mt=mt: v_round(mt) for mt in range(12, 16)]
                + mk_y_chunks(0, 0, 2),
                (3, 0): mk_y_chunks(0, 2, 8) + mk_y_chunks(1, 0, 4),
                (3, 1): mk_y_chunks(1, 4, 8) + mk_y_chunks(2, 0, 8),
            }

            # ---- attention units --------------------------------------
            pending_mults = []

            def emit_unit(i, hk):
                nonlocal pending_mults
                # flush the previous unit's deferred normalizes first:
                # their rec inputs are surely ready, and placing them
                # ahead of this unit's recip chain in the DVE stream
                # keeps a lagging chain from head-of-line blocking them
                for args in pending_mults:
                    nc.vector.tensor_mul(*args)
                pending_mults = []

                jmax = 4 * i + 3
                nd = list(range(4 * i))
                diag = list(range(4 * i, 4 * i + 4))
                # diag 3rd..6th: their exp->mask->AV chain gets 4+ steps
                jlist = nd[:2] + diag + nd[2:]
                chunks = list(fillers.get((i, hk), []))
                pts = {}
                op0 = psum_o.tile([P, 512], F32, tag="o")
                op1 = psum_o.tile([P, 512], F32, tag="o")
                ops = [op0, op1]

                def emit_s(j):
                    c0 = P * (j - 4 * i) if j >= 4 * i else 0
                    sp = psum_s.tile([P, 2, 512], F32, tag="s")
                    for half in range(2):
                        hp = 64 * half
                        nc.tensor.matmul(
                            sp[:, half, c0:],
                            kT_sb[hp : hp + 64, hk, ts(j, P)],
                            qT_sb[hp : hp + 64, hk, 512 * i + c0 : 512 * (i + 1)],
                            start=True,
                            stop=True,
                            tile_position=(hp, 0),
                        )
                    pt = ptiles.tile([P, 2, 512], BF, tag="p")
                    if j >= 4 * i:
                        t = j - 4 * i
                        nc.scalar.activation(
                            pt[:, :, P * t :], sp[:, :, P * t :],
                            Act.Exp, scale=0.125,
                        )
                        nc.gpsimd.tensor_mul(
                            pt[:, :, P * t : P * (t + 1)],
                            pt[:, :, P * t : P * (t + 1)],
                            cmask[:],
                        )
                    else:
                        nc.scalar.activation(pt[:], sp[:], Act.Exp, scale=0.125)
                    pts[j] = pt

                def emit_av(pos):
                    j = jlist[pos]
                    c0 = P * (j - 4 * i) if j >= 4 * i else 0
                    for half in range(2):
                        h = 2 * hk + half
                        nc.tensor.matmul(
                            ops[half][0:65, c0:],
                            v_sb[:, j, h, 0:65],
                            pts[j][:, half, c0:],
                            start=(pos == 0),
                            stop=(pos == len(jlist) - 1),
                        )

                # step loop: filler, S(p), AV(p-2). The psum_s 2-ring makes
                # S(p) wait on exp(p-2), so AV(p-2)'s input is provably
                # ready when it issues.
                n = len(jlist)
                for p in range(n):
                    if p > 0 and chunks:
                        chunks.pop(0)()
                    emit_s(jlist[p])
                    if p >= 2:
                        emit_av(p - 2)
                for p in (n - 2, n - 1):
                    if chunks:
                        chunks.pop(0)()
                    emit_av(p)
                while chunks:
                    chunks.pop(0)()

                # evict O^T (unnormalized) + l rows, then normalize.
                lt = ltmp.tile([P, 2, 512], BF, tag="lt")
                for half in range(2):
                    nc.vector.tensor_copy(
                        lt[64:65, half, :], ops[half][64:65, :]
                    )
                nc.vector.tensor_copy(
                    oT_sb[0:64, hk, ts(i, 512)], ops[0][0:64, :]
                )
                nc.vector.tensor_copy(
                    oT_sb[64:128, hk, ts(i, 512)], ops[1][0:64, :]
                )
                # NOT the psum_p ring: FIFO slot rotation there would
                # make later projection groups wait on this unit's recip
                # chain, serializing the PE filler behind attention
                lb = psum_o.tile([P, 512], F32, tag="o")
                for half in range(2):
                    nc.tensor.matmul(
                        lb[:],
                        sel_t[64:65, half, :],
                        lt[64:65, half, :],
                        start=(half == 0),
                        stop=(half == 1),
                    )
                rec = rbpool.tile([P, 512], F32, tag="rec")
                nc.vector.reciprocal_approx_fast(rec[:], lb[:])
                mult = (
                    oT_sb[:, hk, ts(i, 512)],
                    oT_sb[:, hk, ts(i, 512)],
                    rec[:],
                )
                if i == NQ - 1 and hk == DH // P - 1:
                    nc.vector.tensor_mul(*mult)
                else:
                    pending_mults.append(mult)

            # ---- emission ---------------------------------------------
            qk_round(0, pad=True)
            for mt in range(4):
                v_round(mt, pad=(mt < 2))
            emit_unit(0, 0)
            emit_unit(0, 1)
            for mt in range(4, 8):
                v_round(mt)
            emit_unit(1, 0)
            emit_unit(1, 1)
            for mt in range(10, 12):
                v_round(mt)
            emit_unit(2, 0)
            emit_unit(2, 1)
            emit_unit(3, 0)
            emit_unit(3, 1)

            # tail: Y range 3 on a 4-deep psum ring (psum_p + the now-idle
            # psum_s banks), evictions alternating DVE/ACT. The first 4
            # groups' kc=0 matmuls read only the hk=0 oT block (normalized
            # at unit (3,1) start), so they run while the final normalize
            # chain (lt->lb->recip->mult) is still landing.
            tail_py = []
            for g in range(4):
                mt, n = 12 + g // 2, g % 2
                if g < 2:
                    pg = psum_p.tile([P, 512], F32, tag="pp")
                    py = pg[:]
                else:
                    pg = psum_s.tile([P, 2, 512], F32, tag="s")
                    py = pg[:, 0, :]
                nc.tensor.matmul(
                    py[:],
                    oT_sb[:, 0, ts(mt, P)],
                    wo_sb[:, 0, ts(n, 512)],
                    start=True,
                    stop=False,
                    skip_group_check=True,
                )
                tail_py.append((py, mt, n))
            for g, (py, mt, n) in enumerate(tail_py):
                nc.tensor.matmul(
                    py[:],
                    oT_sb[:, 1, ts(mt, P)],
                    wo_sb[:, 1, ts(n, 512)],
                    start=False,
                    stop=True,
                    skip_group_check=True,
                )
                yt = ytiles.tile([P, 512], BF, tag="y")
                if g % 2:
                    nc.scalar.copy(yt[:], py[:])
                else:
                    nc.vector.tensor_copy(yt[:], py[:])
                nc.sync.dma_start(y_d[ts(mt, P), ts(n, 512)], yt[:])
            for g in range(4, 8):
                mt, n = 12 + g // 2, g % 2
                pool = psum_p if (g // 2) % 2 == 0 else psum_s
                y_group(mt, n, pool=pool, evict="s" if g % 2 else "v")

    nc.compile()
    return nc


def _get_compiled():
    global _compiled
    if _compiled is None:
        _compiled = _build()
    return _compiled


def make_inputs(x, Wq, Wk, Wv, Wo):
    """Shard the full inputs into the 8 per-core input maps (host-side prep)."""
    import ml_dtypes

    bf16 = ml_dtypes.bfloat16
    x = np.asarray(x)
    in_maps = []
    for c in range(N_CORES):
        b, g = divmod(c, HPC)
        rows = slice(g * DH, (g + 1) * DH)
        in_maps.append(
            {
                "xT": np.ascontiguousarray(x[b].T).astype(bf16),
                "wqT": np.ascontiguousarray(np.asarray(Wq)[rows, :].T).astype(bf16),
                "wkT": np.ascontiguousarray(np.asarray(Wk)[rows, :].T).astype(bf16),
                "wvT": np.ascontiguousarray(np.asarray(Wv)[rows, :].T).astype(bf16),
                "woT": np.ascontiguousarray(np.asarray(Wo)[:, rows].T).astype(bf16),
            }
        )
    return in_maps


def assemble(results):
    """Sum the 4 tensor-parallel partials per batch into the full output."""
    y = np.zeros((B, T, C), dtype=np.float32)
    for c in range(N_CORES):
        b = c // HPC
        y[b] += np.asarray(results[c]["y"]).astype(np.float32)
    return y


def kernel(x, Wq, Wk, Wv, Wo):
    from concourse.bass_utils import run_bass_kernel_spmd

    nc = _get_compiled()
    in_maps = make_inputs(x, Wq, Wk, Wv, Wo)
    res = run_bass_kernel_spmd(nc, in_maps, list(range(N_CORES)))
    return assemble(res.results)
